# revision 1
# baseline (speedup 1.0000x reference)
"""Sobel filter Trainium2 Bass kernel.

Problem: img [32, 3, 512, 512] f32, kx/ky [1, 3, 3, 3] f32. Output
[32, 1, 512, 512] f32:
    Gx = valid_conv3x3(img, kx), Gy = valid_conv3x3(img, ky)  -> [N,1,510,510]
    out = sqrt(Gx^2 + Gy^2) edge-padded by 1 back to [N,1,512,512]

Pure data parallel over 8 NeuronCores, 4 images per core. The problem is
memory-regime: per-core mandatory HBM traffic dominates. Fast path (taken for
the actual Sobel kernels, which are rank-1 with shared channel weights):

  * fp16 end to end. Images are cast to fp16 on host (halves load traffic)
    into a [c, y, n, x] per-core layout so every DMA descriptor is a 2KB+
    contiguous run (1KB descriptors measured ~3x below peak HBM bandwidth).
    The device computes in fp16 with fp32 PSUM, stores the magnitude as
    fp16 into a [y, n, x] layout (2KB store descriptors), host upcasts.
    Total rel err ~1e-3 vs the 2e-2 gate.
  * Channel presum T = img_c0 + r1*img_c1 + r2*img_c2 as two DVE 2x-mode
    tensor-adds: cuts the PE stream count from 18 to 6 matmuls per tile.
  * The group whose x-taps have FEWER nonzeros (kx: [1,0,-1] -> 2 taps)
    does its x-conv on the PE: one banded y-conv matmul per nonzero x-tap,
    moving operand x-shifted, accumulating in PSUM.
  * The other group (ky: x-taps [1,2,1]) does its x-conv on T BEFORE the
    matmul (x- and y-convs commute): for binomial taps it is two cascaded
    DVE pair-adds. Both groups' PSUM then holds final Gx/Gy, so ScalarE
    `square` is a fused drain+square; no slow STT ops anywhere (STT has no
    DVE fast modes) and GPSIMD (no PSUM port, 0.42-efficiency ALU) is kept
    off the critical path.
  * Tiles: 4 row-supertiles x 2 image-pairs (ops span 1024 columns,
    amortizing fixed per-op overheads; PSUM = 2 banks per tile, double
    buffered = all 8 banks) plus one "mini" tile with the last rows of all
    4 images in the partition dim. Top/bottom edge-row duplication is
    folded into the stationary matrices (duplicated band columns); x-edge
    duplication is one strided 2-element copy.
  * Emission is software-pipelined two units deep (stage_b(k-2) emitted
    after stage_a(k)): every engine's in-order queue sees a tile's late
    epilogue ops only after later tiles' early ops, so the queues never
    head-of-line block on unfinished dependencies. Loads ride the sync
    (SP) DGE ring, stores the scalar ring -- sharing one ring lets a
    store whose data isn't ready yet block later loads.

Measured engine costs this kernel is balanced against (per [*,1020] op):
DVE TT 2x-mode ~670ns, DVE TS 4x-mode ~410ns, STT 1x ~1330ns (banned),
ACT ~1070ns (dtype-independent), GPSIMD TT ~2100ns with 2-5x outliers,
matmul ~510 cols x ~0.8ns + ~200ns LDWEIGHTS, DMA ~290GB/s at 2KB descs.

Fallback for non-rank-1 kernels: the original generic fp32r path (18 banded
matmuls per tile, full 2D conv on PE).
"""

import os

import numpy as np

N_CORES = 8
N_FULL = 32          # full batch
N_PER_CORE = N_FULL // N_CORES
H = W = 512
NW = 510             # valid output columns

# fast-path tiling: 4 row-supertiles x 2 image-pairs + one mini tile
# (valid-conv row v_j reads input rows j..j+2; out row o>=1 is v_{o-1})
SUP_Y0 = [0, 126, 252, 378]   # input row start per t (o0-1 for t>=1)
SUP_O0 = [0, 127, 253, 379]   # output row start per t
SUP_M = [127, 126, 126, 126]  # output rows per t (t0 col 0 dups v0 = top pad)
MINI_Y0 = 504                 # mini input rows 504..511 (8 per image)
MINI_O0 = 505                 # mini output rows 505..511 (7 per image, incl
MINI_MI = 7                   # bottom pad dup)
MINI_KI = 8

# generic-path constants (baseline fp32r kernel)
TILE_K = 128
TILE_M = 126
N_TILES = 4
MINI_K = 8 * N_PER_CORE
MINI_M = 6 * N_PER_CORE

_CACHE: dict = {}
LAST_RESULTS = None  # BassKernelResults of the most recent run (for test.py)


# ---------------------------------------------------------------------------
# Fast path: rank-1 kernels with shared channel weights (the actual Sobel).
# ---------------------------------------------------------------------------


def _band_std(b):
    """[128, 126]: col m computes valid row v (taps at rows m..m+2)."""
    A = np.zeros((128, 126), np.float64)
    m = np.arange(126)
    for dy in range(3):
        A[m + dy, m] = b[dy]
    return A


def _band_t0(b):
    """[128, 127]: col 0 duplicates v0 (top edge pad), col m+1 = v_m."""
    A = np.zeros((128, 127), np.float64)
    A[0:3, 0] = b
    m = np.arange(126)
    for dy in range(3):
        A[m + dy, m + 1] = b[dy]
    return A


def _band_mini(b):
    """[32, 28] block matrix: input partition 8n+j (image n, row 504+j),
    output col 4r+n (out row 505+r of image n; row-major so the mini store
    lands as one [7, 4, 512] DMA into the y-major out tensor). Col r<=5
    computes v504+r, col r=6 duplicates v509 (bottom edge pad)."""
    A = np.zeros((32, 28), np.float64)
    for n in range(N_PER_CORE):
        for r in range(6):
            for dy in range(3):
                A[MINI_KI * n + r + dy, 4 * r + n] = b[dy]
        for dy in range(3):
            A[MINI_KI * n + 5 + dy, 24 + n] = b[dy]
    return A


def _try_fast(kx, ky):
    """Detect k[g,c,dy,dx] = r_c * b_g[dy] * g_g[dx] structure (shared
    channel ratios, each kernel separable). Returns (stat, statm, cfg, key)
    or None."""
    k = np.stack([np.asarray(kx, np.float64)[0], np.asarray(ky, np.float64)[0]])
    scale = np.abs(k).max()
    if scale == 0:
        return None
    tol = 1e-6 * scale

    # shared channel ratios r_c (relative to reference channel c0)
    c0 = int(np.argmax(np.abs(k).sum(axis=(0, 2, 3))))
    base = k[:, c0]  # [2, 3, 3] (g, dy, dx)
    denom = float((base * base).sum())
    if denom == 0:
        return None
    rc = np.einsum("gyx,gyx->", k[:, c0], base)  # placeholder; per-channel below
    rc = np.array(
        [float(np.einsum("gyx,gyx->", k[:, c], base)) / denom for c in range(3)]
    )
    for c in range(3):
        if np.abs(rc[c] * base - k[:, c]).max() > tol:
            return None

    # per group: nonzero x-tap columns, all parallel (separability)
    groups = []
    for g in range(2):
        B = base[g]  # [dy, dx]
        norms = np.sqrt((B * B).sum(axis=0))
        nz = [dx for dx in range(3) if norms[dx] > 1e-9 * scale]
        if not nz:
            return None
        d0 = nz[int(np.argmax([norms[d] for d in nz]))]
        bd0 = B[:, d0]
        lam = {}
        for d in nz:
            lam[d] = float(B[:, d] @ bd0) / float(bd0 @ bd0)
            if np.abs(lam[d] * bd0 - B[:, d]).max() > tol:
                return None
        groups.append({"nz": nz, "d0": d0, "lam": lam, "B": B})

    # PE group = fewer nonzero x-taps
    P = 0 if len(groups[0]["nz"]) <= len(groups[1]["nz"]) else 1
    V = 1 - P
    gP, gV = groups[P], groups[V]
    P_dxs = gP["nz"]
    nP = len(P_dxs)
    n_sl = nP + 1

    # stationary bands: P group uses each nonzero column directly (exact);
    # V group uses its d0 column. With a symmetric 3-tap V x-kernel the
    # x-conv runs on T pre-matmul as (T[dA]+T[dB]) + (1/lam)T[d0], so the
    # V stationary carries the lam scale.
    P_bands = [gP["B"][:, dx] for dx in P_dxs]
    V_band = gV["B"][:, gV["d0"]]
    V_rest = [(d, gV["lam"][d]) for d in gV["nz"] if d != gV["d0"]]
    if len(V_rest) == 2 and V_rest[0][1] == V_rest[1][1] and V_rest[0][1] != 0.0:
        V_band = V_band * V_rest[0][1]

    bands = P_bands + [V_band]
    stat = np.zeros((128, n_sl * 2 * 128), np.float64)
    for i, b in enumerate(bands):
        stat[:, 128 * i : 128 * i + 127] = _band_t0(b)
        off = 128 * (n_sl + i)
        stat[:, off : off + 126] = _band_std(b)
    statm = np.zeros((32, n_sl * 28), np.float64)
    for i, b in enumerate(bands):
        statm[:, 28 * i : 28 * (i + 1)] = _band_mini(b)

    chan = [(c, float(rc[c])) for c in range(3) if c != c0]
    # symmetric 3-tap V kernel (lam equal on the two non-center taps):
    # enables the scale-folded TT/TS x-conv chain (no slow STT ops)
    V_sym = None
    if len(V_rest) == 2 and V_rest[0][1] == V_rest[1][1] and V_rest[0][1] != 0.0:
        V_sym = (V_rest[0][1], V_rest[0][0], V_rest[1][0])
    cfg = {
        "n_sl": n_sl,
        "P_dxs": P_dxs,
        "V_d0": gV["d0"],
        "V_rest": V_rest,
        "V_sym": V_sym,
        "c0": c0,
        "chan": chan,
        # equal channel weights -> presum can ride on accumulating DMAs
        "presum_dma": all(r == 1.0 for _, r in chan),
    }
    key = (
        "fast",
        tuple(P_dxs),
        gV["d0"],
        tuple((d, round(l, 12)) for d, l in V_rest),
        c0,
        tuple((c, round(r, 12)) for c, r in chan),
    )
    return (
        np.ascontiguousarray(stat.astype(np.float16)),
        np.ascontiguousarray(statm.astype(np.float16)),
        cfg,
        key,
    )


def _sobel_body_fast(tc, out, img, stat_dram, statm_dram, cfg):
    """img dram [3, 512, 4, 512] fp16 (c, y, n, x -- 2KB-contiguous rows per
    (c,y): DMA descriptors below 2KB run far below peak HBM bandwidth).
    out dram [512, 4, 512] fp16 (y, n, x -- image-pair stores are one 2KB
    descriptor per partition)."""
    import concourse.mybir as mybir

    nc = tc.nc
    f32 = mybir.dt.float32
    f16 = mybir.dt.float16
    AL = mybir.AluOpType

    nP = len(cfg["P_dxs"])
    n_sl = cfg["n_sl"]
    c0 = cfg["c0"]
    (cA, rA), (cB, rB) = cfg["chan"]
    Vd0 = cfg["V_d0"]
    V_rest = cfg["V_rest"]
    V_sym = cfg["V_sym"]

    img_ny = img.rearrange("c y n x -> c n y x")  # [3, 4, 512, 512]

    presum_dma = cfg["presum_dma"] and os.environ.get("SOBEL_PRESUM", "") == "dma"

    with (
        tc.tile_pool(name="const", bufs=1) as const_pool,
        tc.tile_pool(name="imgs", bufs=6) as img_pool,
        tc.tile_pool(name="work", bufs=6) as work_pool,
        tc.tile_pool(name="psum", bufs=2, space="PSUM") as psum_pool,
    ):
        stat_sb = const_pool.tile([128, n_sl * 2 * 128], f16)
        nc.sync.dma_start(out=stat_sb, in_=stat_dram)
        statm_sb = const_pool.tile([32, n_sl * 28], f16)
        nc.sync.dma_start(out=statm_sb, in_=statm_dram)

        def presum_dve(it, K, F):
            """T = i_c0 + rA*i_cA + rB*i_cB from a loaded 3ch tile. For unit
            ratios (Sobel) two 2x-mode TT adds; else STT (slow fallback)."""
            T = work_pool.tile([K, F, W], f16, tag="T", name="T", bufs=4)
            if rA == 1.0 and rB == 1.0:
                nc.vector.tensor_add(T, it[:, cA], it[:, c0])
                nc.vector.tensor_add(T, T, it[:, cB])
            else:
                nc.vector.scalar_tensor_tensor(
                    T, it[:, cA], rA, it[:, c0], AL.mult, AL.add
                )
                nc.vector.scalar_tensor_tensor(
                    T, it[:, cB], rB, T, AL.mult, AL.add
                )
            return T

        def make_tv(T, K, F):
            """V-group x-conv applied to T (x- and y-convs commute):
            Tv = (T[dA]+T[dB]) + (1/lam)*T[d0]; the V stationary carries the
            lam scale. Valid Tv cols 0..509."""
            binom = (
                V_sym is not None
                and V_sym[0] == 0.5
                and Vd0 == 1
                and sorted((V_sym[1], V_sym[2])) == [0, 2]
            )
            if binom:
                # [1,2,1]-proportional taps: two cascaded pair-adds
                u = work_pool.tile([K, F, W], f16, tag="tv1", name="u")
                nc.vector.tensor_add(
                    u[:, :, 0 : W - 1], T[:, :, 0 : W - 1], T[:, :, 1:W]
                )
                Tv = work_pool.tile([K, F, W], f16, tag="Tv", name="Tv", bufs=4)
                nc.vector.tensor_add(
                    Tv[:, :, 0:NW], u[:, :, 0:NW], u[:, :, 1 : 1 + NW]
                )
                return Tv
            if V_sym:
                lam, dA, dB = V_sym
                tv1 = work_pool.tile([K, F, W], f16, tag="tv1", name="tv1")
                nc.vector.tensor_add(
                    tv1[:, :, 0:NW], T[:, :, dA : dA + NW], T[:, :, dB : dB + NW]
                )
                tv2 = work_pool.tile([K, F, W], f16, tag="tv2", name="tv2")
                nc.vector.tensor_scalar_mul(
                    tv2[:, :, 0:NW], T[:, :, Vd0 : Vd0 + NW], 1.0 / lam
                )
                Tv = work_pool.tile([K, F, W], f16, tag="Tv", name="Tv", bufs=4)
                nc.vector.tensor_add(
                    Tv[:, :, 0:NW], tv1[:, :, 0:NW], tv2[:, :, 0:NW]
                )
                return Tv
            # generic taps: STT chain (slower; non-Sobel shapes only)
            acc = T[:, :, Vd0 : Vd0 + NW]
            Tv = work_pool.tile([K, F, W], f16, tag="Tv", name="Tv", bufs=4)
            first = True
            for d, lam in V_rest:
                nc.vector.scalar_tensor_tensor(
                    Tv[:, :, 0:NW], T[:, :, d : d + NW], lam, acc,
                    AL.mult, AL.add,
                )
                acc = Tv[:, :, 0:NW]
                first = False
            if first:
                nc.vector.tensor_copy(Tv[:, :, 0:NW], acc)
            return Tv

        def stage_a(T, M, F, stat, offs):
            """x-conv + MMs + PSUM square-drains; returns (s, sq2) tiles."""
            Tv = make_tv(T, T.shape[0], F)
            # P group: full conv via x-shifted accumulating matmuls, one
            # matmul per (image, tap) -- strided 2D moving APs and >512-f32
            # PSUM rows are both ISA-illegal.
            S1 = psum_pool.tile([M, F, W], f32, tag="S1", name="S1", bufs=2)
            for p in range(F):
                for i, dx in enumerate(cfg["P_dxs"]):
                    nc.tensor.matmul(
                        S1[:, p, 0:NW],
                        stat[:, offs[i] : offs[i] + M],
                        T[:, p, dx : dx + NW],
                        start=(i == 0),
                        stop=(i == nP - 1),
                    )
            # V group: y-conv of the x-convolved Tv -> PSUM holds final Gy
            S2 = psum_pool.tile([M, F, W], f32, tag="S2", name="S2", bufs=2)
            for p in range(F):
                nc.tensor.matmul(
                    S2[:, p, 0:NW],
                    stat[:, offs[nP] : offs[nP] + M],
                    Tv[:, p, 0:NW],
                    start=True, stop=True,
                )
            # ScalarE: drain+square both groups straight out of PSUM
            s = work_pool.tile([M, F, W], f16, tag="s", name="s", bufs=4)
            nc.scalar.square(s[:, :, 1 : 1 + NW], S1[:, :, 0:NW])
            sq2 = work_pool.tile([M, F, W], f16, tag="sq2", name="sq2", bufs=4)
            nc.scalar.square(sq2[:, :, 0:NW], S2[:, :, 0:NW])
            return s, sq2

        store_ctr = [0]

        def stage_b(s, sq2, M, F, out_ap, unit):
            """Combine squares + magnitude + store for one tile."""
            nc.vector.tensor_add(
                s[:, :, 1 : 1 + NW], s[:, :, 1 : 1 + NW], sq2[:, :, 0:NW]
            )
            # x edge pad columns: both edges in one strided copy (on the
            # otherwise-idle GPSIMD)
            nc.gpsimd.tensor_copy(
                s[:, :, 0 : W : W - 1], s[:, :, 1 : W - 1 : W - 3]
            )
            mag = work_pool.tile([M, F, W], f16, tag="mag", name="mag")
            nc.scalar.sqrt(mag, s)
            # alternate store rings: halves ACT-seq config occupancy (ACT's
            # exec queue depth is 0, so configs stall its engine) while only
            # every other store can head-of-line delay the load ring
            ring = nc.scalar if store_ctr[0] % 2 == 0 else nc.sync
            store_ctr[0] += 1
            ring.dma_start(out=out_ap, in_=mag)

        offs_t0 = [128 * i for i in range(n_sl)]
        offs_std = [128 * (n_sl + i) for i in range(n_sl)]
        offs_m = [28 * i for i in range(n_sl)]

        def emit_mini_t():
            """Mini loads + presum: mit [32, 3, 512] -> Tm [32, 1, 512]."""
            mit = img_pool.tile([32, 3, W], f16, tag="mit", name="mit", bufs=1)
            for c in range(3):
                nc.sync.dma_start(
                    out=mit[:, c],
                    in_=img_ny[c, :, MINI_Y0 : MINI_Y0 + MINI_KI],
                )
            Tm = work_pool.tile([32, 1, W], f16, tag="T", name="Tm", bufs=4)
            if rA == 1.0 and rB == 1.0:
                nc.vector.tensor_add(Tm[:, 0], mit[:, cA], mit[:, c0])
                nc.vector.tensor_add(Tm[:, 0], Tm[:, 0], mit[:, cB])
            else:
                nc.vector.scalar_tensor_tensor(
                    Tm[:, 0], mit[:, cA], rA, mit[:, c0], AL.mult, AL.add
                )
                nc.vector.scalar_tensor_tensor(
                    Tm[:, 0], mit[:, cB], rB, Tm[:, 0], AL.mult, AL.add
                )
            return Tm

        # Two-stage software pipeline with depth-2 stagger: emit stage_b(k-2)
        # alongside stage_a(k), so each engine's in-order queue sees tile
        # k-2's late epilogue only after tile k's early MM/drain work -- by
        # then its dependencies have long completed and nothing stalls.
        # First super ahead of the mini so its (bigger) loads start streaming
        # immediately.
        units = [(0, 0), ("mini", None), (0, 1)] + [
            (t, p) for t in range(1, 4) for p in range(2)
        ]

        def emit_a(unit, si):
            t, p = unit
            if t == "mini":
                Tm = emit_mini_t()
                s, sq2 = stage_a(Tm, N_PER_CORE * MINI_MI, 1, statm_sb,
                                 offs_m)
                return (s, sq2, N_PER_CORE * MINI_MI, 1,
                        out[MINI_O0:H, :, :], unit)
            offs = offs_t0 if t == 0 else offs_std
            y0, o0, m = SUP_Y0[t], SUP_O0[t], SUP_M[t]
            if presum_dma:
                # channel presum rides on accumulating SWDGE loads
                T = work_pool.tile([128, 2, W], f16, tag="T", name="T",
                                   bufs=4)
                nc.sync.dma_start(
                    out=T, in_=img[c0, y0 : y0 + 128, 2 * p : 2 * p + 2]
                )
                for c, _r in cfg["chan"]:
                    nc.gpsimd.dma_start(
                        out=T,
                        in_=img[c, y0 : y0 + 128, 2 * p : 2 * p + 2],
                        accum_op=AL.add,
                    )
            else:
                it = img_pool.tile([128, 3, 2, W], f16, tag="it", name="it")
                for c in range(3):
                    nc.sync.dma_start(
                        out=it[:, c],
                        in_=img[c, y0 : y0 + 128, 2 * p : 2 * p + 2],
                    )
                T = presum_dve(it, 128, 2)
            s, sq2 = stage_a(T, m, 2, stat_sb, offs)
            return (s, sq2, m, 2, out[o0 : o0 + m, 2 * p : 2 * p + 2, :], unit)

        from collections import deque

        pending = deque()
        for si, unit in enumerate(units):
            pending.append(emit_a(unit, si))
            if len(pending) > 2:
                stage_b(*pending.popleft())
        while pending:
            stage_b(*pending.popleft())


def _build_program_fast(cfg):
    import concourse.bacc as bacc
    import concourse.mybir as mybir
    import concourse.tile as tile

    nc = bacc.Bacc(
        "TRN2", target_bir_lowering=False, debug=False, num_devices=N_CORES
    )
    n_sl = cfg["n_sl"]
    img = nc.dram_tensor(
        "img", [3, H, N_PER_CORE, W], mybir.dt.float16, kind="ExternalInput"
    ).ap()
    stat = nc.dram_tensor(
        "stat", [128, n_sl * 2 * 128], mybir.dt.float16, kind="ExternalInput"
    ).ap()
    statm = nc.dram_tensor(
        "statm", [32, n_sl * 28], mybir.dt.float16, kind="ExternalInput"
    ).ap()
    out = nc.dram_tensor(
        "out", [H, N_PER_CORE, W], mybir.dt.float16, kind="ExternalOutput"
    ).ap()
    with tile.TileContext(nc) as tc:
        _sobel_body_fast(tc, out, img, stat, statm, cfg)
    nc.compile()
    return nc


# ---------------------------------------------------------------------------
# Generic fallback (arbitrary kx/ky): full 2D conv as 18 banded fp32r
# matmuls per tile. Unchanged from the baseline kernel.
# ---------------------------------------------------------------------------


def _build_stationaries(kx: np.ndarray, ky: np.ndarray):
    """Returns (stat [TILE_K, 18, TILE_M], stat_mini [MINI_K, 18, MINI_M]).
    Slice i=(g,c,dx) of stat is the banded matrix A[k, m] = kG[c, k-m, dx]
    for k-m in {0,1,2}; stat_mini is block-diagonal per image."""
    ks = (np.asarray(kx, np.float32), np.asarray(ky, np.float32))
    stat = np.zeros((18, TILE_K, TILE_M), np.float32)
    mini = np.zeros((18, MINI_K, MINI_M), np.float32)
    m = np.arange(TILE_M)
    mm = np.arange(6)
    i = 0
    for g in range(2):
        for c in range(3):
            for dx in range(3):
                for dy in range(3):
                    stat[i, m + dy, m] = ks[g][0, c, dy, dx]
                    for j in range(N_PER_CORE):
                        mini[i, j * 8 + mm + dy, j * 6 + mm] = ks[g][0, c, dy, dx]
                i += 1
    return (
        np.ascontiguousarray(stat.transpose(1, 0, 2)),
        np.ascontiguousarray(mini.transpose(1, 0, 2)),
    )


def _epilogue(nc, work_pool, psx, psy, rows, f32):
    """sqrt(psx^2 + psy^2) -> [rows, 512] SBUF tile with edge cols."""
    s = work_pool.tile([rows, W], f32, tag="s", name="s")
    s2 = work_pool.tile([rows, NW], f32, tag="s2", name="s2")
    nc.scalar.square(s[:, 1 : 1 + NW], psx)
    nc.scalar.square(s2, psy)
    nc.vector.tensor_add(s[:, 1 : 1 + NW], s[:, 1 : 1 + NW], s2)
    nc.vector.tensor_copy(s[:, 0:1], s[:, 1:2])
    nc.vector.tensor_copy(s[:, W - 1 : W], s[:, W - 2 : W - 1])
    mag = work_pool.tile([rows, W], f32, tag="mag", name="mag")
    nc.scalar.sqrt(mag, s)
    return mag


def _sobel_body(tc, out, img, stat_dram, stat_mini_dram):
    import concourse.mybir as mybir

    nc = tc.nc
    f32 = mybir.dt.float32
    mm_dt = mybir.dt.float32r

    img_yx = img.rearrange("n c y x -> n y c x")

    with (
        tc.tile_pool(name="const", bufs=1) as const_pool,
        tc.tile_pool(name="imgs", bufs=3) as img_pool,
        tc.tile_pool(name="work", bufs=4) as work_pool,
        tc.tile_pool(name="psum", bufs=2, space="PSUM") as psum_pool,
    ):
        stat_mini_sb = const_pool.tile([MINI_K, 18, MINI_M], mm_dt)
        nc.sync.dma_start(out=stat_mini_sb, in_=stat_mini_dram)
        mit = img_pool.tile([MINI_K, 3, W], mm_dt, tag="mit", bufs=1)
        for c in range(3):
            nc.sync.dma_start(out=mit[:, c, :], in_=img_yx[:, H - 8 : H, c])
        stat_sb = const_pool.tile([TILE_K, 18, TILE_M], mm_dt)
        for j in range(5):
            nc.sync.dma_start(
                out=stat_sb[:, 2 * j : 2 * j + 2], in_=stat_dram[:, 2 * j : 2 * j + 2]
            )
        for j in range(5, 9):
            nc.scalar.dma_start(
                out=stat_sb[:, 2 * j : 2 * j + 2], in_=stat_dram[:, 2 * j : 2 * j + 2]
            )

        def big_tile(n, t):
            y0 = t * TILE_M
            its = []
            for c in range(3):
                itc = img_pool.tile(
                    [TILE_K, W], mm_dt, tag=f"it{c}", name=f"it{c}", bufs=6
                )
                nc.sync.dma_start(out=itc, in_=img_yx[n, y0 : y0 + TILE_K, c])
                its.append(itc)

            psx = psum_pool.tile([TILE_M, NW], f32, tag="psx", name="psx")
            psy = psum_pool.tile([TILE_M, NW], f32, tag="psy", name="psy")
            for g, ps in ((0, psx), (1, psy)):
                mmi = 0
                for c in range(3):
                    for dx in range(3):
                        i = (g * 3 + c) * 3 + dx
                        nc.tensor.matmul(
                            ps,
                            stat_sb[:, i, :],
                            its[c][:, dx : dx + NW],
                            start=(mmi == 0),
                            stop=(mmi == 8),
                        )
                        mmi += 1

            mag = _epilogue(nc, work_pool, psx, psy, TILE_M, f32)
            nc.scalar.dma_start(out=out[n, 1 + y0 : 1 + y0 + TILE_M, :], in_=mag)
            if t == 0:
                nc.scalar.dma_start(out=out[n, 0:1, :], in_=mag[0:1, :])

        def mini_tile():
            mpsx = psum_pool.tile([MINI_M, NW], f32, tag="mpsx", bufs=1, name="mpsx")
            mpsy = psum_pool.tile([MINI_M, NW], f32, tag="mpsy", bufs=1, name="mpsy")
            for g, ps in ((0, mpsx), (1, mpsy)):
                mmi = 0
                for c in range(3):
                    for dx in range(3):
                        i = (g * 3 + c) * 3 + dx
                        nc.tensor.matmul(
                            ps,
                            stat_mini_sb[:, i, :],
                            mit[:, c, dx : dx + NW],
                            start=(mmi == 0),
                            stop=(mmi == 8),
                        )
                        mmi += 1
            mmag = _epilogue(nc, work_pool, mpsx, mpsy, MINI_M, f32)
            for n in range(N_PER_CORE):
                nc.scalar.dma_start(
                    out=out[n, H - 7 : H - 1, :], in_=mmag[n * 6 : n * 6 + 6]
                )
                nc.scalar.dma_start(
                    out=out[n, H - 1 : H, :], in_=mmag[n * 6 + 5 : n * 6 + 6]
                )

        mini_tile()
        for n in range(N_PER_CORE):
            for t in range(N_TILES):
                big_tile(n, t)


def _build_program():
    import concourse.bacc as bacc
    import concourse.mybir as mybir
    import concourse.tile as tile

    nc = bacc.Bacc(
        "TRN2",
        target_bir_lowering=False,
        debug=False,
        num_devices=N_CORES,
    )
    img = nc.dram_tensor(
        "img", [N_PER_CORE, 3, H, W], mybir.dt.float32r, kind="ExternalInput"
    ).ap()
    stat = nc.dram_tensor(
        "stat", [TILE_K, 18, TILE_M], mybir.dt.float32r, kind="ExternalInput"
    ).ap()
    stat_mini = nc.dram_tensor(
        "stat_mini", [MINI_K, 18, MINI_M], mybir.dt.float32r, kind="ExternalInput"
    ).ap()
    out = nc.dram_tensor(
        "out", [N_PER_CORE, H, W], mybir.dt.float32, kind="ExternalOutput"
    ).ap()

    with tile.TileContext(nc) as tc:
        _sobel_body(tc, out, img, stat, stat_mini)
    nc.compile()
    return nc


def _run(nc, in_maps):
    global LAST_RESULTS
    from concourse.bass_utils import run_bass_kernel_spmd

    trace = os.environ.get("SOBEL_TRACE", "0") == "1"
    res = run_bass_kernel_spmd(
        nc, in_maps, core_ids=list(range(N_CORES)), trace=trace
    )
    LAST_RESULTS = res
    return np.concatenate([res.results[c]["out"] for c in range(N_CORES)], axis=0)


def kernel(img: np.ndarray, kx: np.ndarray, ky: np.ndarray) -> np.ndarray:
    img = np.ascontiguousarray(np.asarray(img, dtype=np.float32))
    assert img.shape == (N_FULL, 3, H, W), img.shape

    fast = (
        _try_fast(kx, ky)
        if os.environ.get("SOBEL_NO_FAST", "0") != "1"
        else None
    )
    if fast is not None:
        stat, statm, cfg, key = fast
        key = key + (
            os.environ.get("SOBEL_PRESUM", ""),
            os.environ.get("SOBEL_DRAIN", "act"),
        )
        if key not in _CACHE:
            _CACHE[key] = _build_program_fast(cfg)
        nc = _CACHE[key]
        # per-core [c, y, n, x] fp16 layout: every (c, y) row pair is a
        # 2KB-contiguous DMA descriptor on device
        img16 = img.astype(np.float16)
        in_maps = [
            {
                "img": np.ascontiguousarray(
                    img16[c * N_PER_CORE : (c + 1) * N_PER_CORE].transpose(
                        1, 2, 0, 3
                    )
                ),
                "stat": stat,
                "statm": statm,
            }
            for c in range(N_CORES)
        ]
        out = _run(nc, in_maps)  # [8 cores x [512, 4, 512]] fp16, y-major
        out = out.reshape(N_CORES, H, N_PER_CORE, W).transpose(0, 2, 1, 3)
        return (
            np.ascontiguousarray(out)
            .reshape(N_FULL, 1, H, W)
            .astype(np.float32)
        )

    stat, stat_mini = _build_stationaries(kx, ky)
    if "gen" not in _CACHE:
        _CACHE["gen"] = _build_program()
    nc = _CACHE["gen"]
    in_maps = [
        {
            "img": img[c * N_PER_CORE : (c + 1) * N_PER_CORE],
            "stat": stat,
            "stat_mini": stat_mini,
        }
        for c in range(N_CORES)
    ]
    out = _run(nc, in_maps)
    return out.reshape(N_FULL, 1, H, W)

